# revision 28
# baseline (speedup 1.0000x reference)
"""Trainium2 Bass kernel for GQA attention with RoPE (dense_transformer).

Model: B=2, T=2048, C=2048, H=16 query heads, KV=4 kv heads, D=128, causal.
Sharding: 8 cores = batch(2) x kv-group(4) tensor parallel. Each core computes
its batch's 4 query heads (one kv head), then a partial output projection over
its 512 head-dims; per-group ReduceScatters (4 cores per batch) sum the
partials. The host reassembles the stripes.

v2 layout/schedule:
 - All inputs are pre-cast to bf16 and pre-transposed on the host: x arrives
   as xT chunk-major, the qkv weights arrive transposed+concatenated (wq/wk
   rows permuted even/odd per head for rope), wo arrives transposed, and the
   rope cos/sin tables arrive as ready [128, T] operands with the sign and
   1/sqrt(D) folded in. The device does no f32 casts and no weight/x
   transposes.
 - Projection chunks and attention groups are interleaved (group g only needs
   K/V chunks 0..g and Q chunk g), so the per-group out-projection +
   ReduceScatter chain starts early and overlaps compute; only the last
   group's RS is exposed at the end.
 - RoPE is 4 vector ops per tensor-chunk using cross-partition-base reads.
 - Softmax: scoresT [tk, tq] via PE; exp on ACT evicts PSUM->SBUF; sums via
   ones-matmul rows 0/1; reciprocal on DVE straight from PSUM; broadcast via
   GpSimd partition_broadcast; the normalize multiply also evicts the PV
   accumulator (DVE reads PSUM), so no PE/ACT cycles go to normalization.
"""

import os

os.environ.setdefault("MYCRO_LOCAL_CACHE", "1")

import numpy as np

B, T, C = 2, 2048, 2048
H, KV, D = 16, 4, 128
HL = H // KV          # 4 local query heads per core
NCORES = 8
P = 128
SCALE = 1.0 / float(np.sqrt(D))

NCC = C // P          # 16 contraction tiles
NCH = T // 512        # 4 t-chunks of 512
NG = NCH              # 4 query groups (512 queries each)
TQ = 512
NEG = -1e10


def _emit(nc, tile, mybir, ExitStack):
    f32 = mybir.dt.float32
    bf16 = mybir.dt.bfloat16
    Exp = mybir.ActivationFunctionType.Exp
    Copy = mybir.ActivationFunctionType.Copy
    add = mybir.AluOpType.add
    mult = mybir.AluOpType.mult

    xt4 = nc.dram_tensor("xt4", [NCH * C, 512], bf16, kind="ExternalInput")
    wqkv = nc.dram_tensor("wqkv", [C, (HL + 2) * P], bf16, kind="ExternalInput")
    wot = nc.dram_tensor("wot", [HL * P, C], bf16, kind="ExternalInput")
    raq = nc.dram_tensor("raq", [P, T], bf16, kind="ExternalInput")
    rbq = nc.dram_tensor("rbq", [P, T], bf16, kind="ExternalInput")
    rak = nc.dram_tensor("rak", [P, T], bf16, kind="ExternalInput")
    rbk = nc.dram_tensor("rbk", [P, T], bf16, kind="ExternalInput")
    cstf = nc.dram_tensor("cstf", [P, P], f32, kind="ExternalInput")
    cstb = nc.dram_tensor("cstb", [P, 2 * P + 2], bf16, kind="ExternalInput")
    out = nc.dram_tensor("out", [T // 4, C], bf16, kind="ExternalOutput")

    te, ve, sc, gp, sy = nc.tensor, nc.vector, nc.scalar, nc.gpsimd, nc.sync

    with tile.TileContext(nc) as tc, ExitStack() as ctx:
        consts = ctx.enter_context(tc.tile_pool(name="consts", bufs=1))
        persist = ctx.enter_context(tc.tile_pool(name="persist", bufs=1))
        dram = ctx.enter_context(tc.tile_pool(name="dram", bufs=1, space="DRAM"))
        sbX = ctx.enter_context(tc.tile_pool(name="sbX", bufs=2))
        sbQK = ctx.enter_context(tc.tile_pool(name="sbQK", bufs=4))
        sbR = ctx.enter_context(tc.tile_pool(name="sbR", bufs=3))
        sbP = ctx.enter_context(tc.tile_pool(name="sbP", bufs=6))
        sbBC = ctx.enter_context(tc.tile_pool(name="sbBC", bufs=4))
        sbRC = ctx.enter_context(tc.tile_pool(name="sbRC", bufs=2))
        sbAT = ctx.enter_context(tc.tile_pool(name="sbAT", bufs=2))
        sbY = ctx.enter_context(tc.tile_pool(name="sbY", bufs=3))
        psF = ctx.enter_context(tc.tile_pool(name="psF", bufs=2, space="PSUM"))
        psS = ctx.enter_context(tc.tile_pool(name="psS", bufs=3, space="PSUM"))
        psA = ctx.enter_context(tc.tile_pool(name="psA", bufs=1, space="PSUM"))
        psB = ctx.enter_context(tc.tile_pool(name="psB", bufs=1, space="PSUM"))
        psD = ctx.enter_context(tc.tile_pool(name="psD", bufs=1, space="PSUM"))

        # constants come in via DMA (GpSimd-computed consts raced their
        # first-run consumers: cold-start ucode made memset/affine_select
        # land late and the first run read garbage)
        ident = consts.tile([P, P], bf16, tag="ident")
        sy.dma_start(ident[:], cstb.ap()[:, 0:P])
        ones = consts.tile([P, 2], bf16, tag="ones")
        sy.dma_start(ones[:], cstb.ap()[:, P:P + 2])
        onesc = consts.tile([P, P], bf16, tag="onesc")
        sy.dma_start(onesc[:], cstb.ap()[:, P + 2:2 * P + 2])
        # scoresT layout [tk, tq]: keep where tq >= tk, else -1e10.
        triT = consts.tile([P, P], f32, tag="triT")
        sy.dma_start(triT[:], cstf.ap()[:, :])

        wqkvT = [persist.tile([P, (HL + 2) * P], bf16, tag=f"wqkvT{cc}",
                              name=f"wqkvT{cc}") for cc in range(NCC)]
        woTs = [persist.tile([P, C], bf16, tag=f"woT{h}", name=f"woT{h}")
                for h in range(HL)]
        tabs = {}
        for nm, src in (("aq", raq), ("bq", rbq), ("ak", rak), ("bk", rbk)):
            tabs[nm] = persist.tile([P, T], bf16, tag=f"tab_{nm}",
                                    name=f"tab_{nm}")
        qrT = [persist.tile([P, T], bf16, tag=f"qrT{h}", name=f"qrT{h}")
               for h in range(HL)]
        krT = persist.tile([P, T], bf16, tag="krT")
        vnat = persist.tile([P, T], bf16, tag="vnat")

        y_dram = [dram.tile([TQ, C], bf16, tag=f"ydram{g}", name=f"ydram{g}")
                  for g in range(NG)]
        rs_out = [dram.tile([64, C], bf16, tag=f"rsout{g}", name=f"rsout{g}")
                  for g in range(2 * NG)]

        # ---- upfront weight/table DMAs. x tiles go on sy/sc, weights on gp,
        # rope tables on the vector engine's queue so nothing delays x; wo is
        # emitted later (first read is outproj(0), far downstream)
        for cc in range(NCC):
            gp.dma_start(wqkvT[cc][:], wqkv.ap()[cc * P:(cc + 1) * P, :])
        for nm, src in (("aq", raq), ("bq", rbq), ("ak", rak), ("bk", rbk)):
            gp.dma_start(tabs[nm][:], src.ap()[:, :])

        def late_loads():
            for h in range(HL):
                gp.dma_start(woTs[h][:], wot.ap()[h * P:(h + 1) * P, :])

        def rope(dst, sl, qs, ta, tb):
            # dst[:,sl] = qs * ta + swap_halves(qs) * tb
            w = sbR.tile([P, 512], bf16, tag="ropew", name="ropew")
            sy.dma_start(w[0:64, :], qs[64:P, :])
            sy.dma_start(w[64:P, :], qs[0:64, :])
            t1 = sbR.tile([P, 512], bf16, tag="ropet1", name="ropet1")
            t2 = sbR.tile([P, 512], bf16, tag="ropet2", name="ropet2")
            ve.tensor_mul(t1[:], qs[:], ta[:, sl])
            ve.tensor_mul(t2[:], w[:], tb[:, sl])
            ve.tensor_add(dst[:, sl], t1[:], t2[:])

        def proj(ch):
            sl = slice(ch * 512, (ch + 1) * 512)
            xts = []
            for cc in range(NCC):
                xt = sbX.tile([P, 512], bf16, tag=f"xT{cc}", name=f"xT{cc}")
                (sy if cc % 2 == 0 else sc).dma_start(
                    xt[:], xt4.ap()[ch * C + cc * P:ch * C + (cc + 1) * P, :])
                xts.append(xt)
            for h in range(HL):
                ps = psF.tile([P, 512], f32, tag="fat", name="fat")
                for cc in range(NCC):
                    te.matmul(ps[:], wqkvT[cc][:, h * P:(h + 1) * P], xts[cc][:],
                              start=(cc == 0), stop=(cc == NCC - 1))
                qs = sbQK.tile([P, 512], bf16, tag="qkev", name="qkev")
                sc.activation(qs[:], ps[:], Copy)
                rope(qrT[h], sl, qs, tabs["aq"], tabs["bq"])
            # k
            ps = psF.tile([P, 512], f32, tag="fat", name="fat")
            for cc in range(NCC):
                te.matmul(ps[:], wqkvT[cc][:, HL * P:(HL + 1) * P], xts[cc][:],
                          start=(cc == 0), stop=(cc == NCC - 1))
            qs = sbQK.tile([P, 512], bf16, tag="qkev", name="qkev")
            sc.activation(qs[:], ps[:], Copy)
            rope(krT, sl, qs, tabs["ak"], tabs["bk"])
            # v -> natural layout [tk, dv] blocks via PE transpose
            ps = psF.tile([P, 512], f32, tag="fat", name="fat")
            for cc in range(NCC):
                te.matmul(ps[:], wqkvT[cc][:, (HL + 1) * P:(HL + 2) * P],
                          xts[cc][:], start=(cc == 0), stop=(cc == NCC - 1))
            vt = sbQK.tile([P, 512], bf16, tag="qkev", name="qkev")
            sc.activation(vt[:], ps[:], Copy)
            pv = psF.tile([P, 512], bf16, tag="fat", name="fat")
            for i in range(4):
                te.transpose(pv[:, i * P:(i + 1) * P], vt[:, i * P:(i + 1) * P],
                             ident[:])
            sc.activation(vnat[:, sl], pv[:], Copy)

        def emit_scores(gq, kb, hs):
            j = kb - 4 * gq
            w0 = max(j, 0) * P
            probs = []
            for h in hs:
                st = psS.tile([P, TQ], f32, tag="score", name="score")
                te.matmul(
                    st[:, w0:TQ],
                    krT[:, kb * P:(kb + 1) * P],
                    qrT[h][:, gq * TQ + w0:(gq + 1) * TQ],
                    start=True, stop=True,
                )
                if j >= 0:
                    ve.tensor_tensor(
                        st[:, w0:w0 + P], st[:, w0:w0 + P], triT[:], add)
                pb = sbP.tile([P, TQ], bf16, tag="probs", name="probs")
                sc.activation(pb[:, w0:TQ], st[:, w0:TQ], Exp)
                probs.append(pb)
            return probs, w0

        def emit_accum(kb, kbmax, w0, probs, pa, psums):
            for i in range(2):
                te.matmul(
                    psums[64 * i:64 * i + 1, w0:TQ], ones[:, i:i + 1],
                    probs[i][:, w0:TQ],
                    start=(kb == 0), stop=(kb == kbmax - 1),
                )
            for i in range(2):
                te.matmul(
                    pa[i][:, w0:TQ], vnat[:, kb * P:(kb + 1) * P],
                    probs[i][:, w0:TQ],
                    start=(kb == 0), stop=(kb == kbmax - 1),
                )

        attn_cur = {}

        def attn(gq):
            kbmax = 4 * (gq + 1)
            for hp in range(HL // 2):
                hs = (2 * hp, 2 * hp + 1)
                pa = [psA.tile([P, TQ], f32, tag="paA", name="paA"),
                      psB.tile([P, TQ], f32, tag="paB", name="paB")]
                psums = psD.tile([P, TQ], f32, tag="psums", name="psums")
                prev = None
                for kb in range(kbmax):
                    cur = (kb, *emit_scores(gq, kb, hs))
                    if prev is not None:
                        pkb, pprobs, pw0 = prev
                        emit_accum(pkb, kbmax, pw0, pprobs, pa, psums)
                    prev = cur
                pkb, pprobs, pw0 = prev
                emit_accum(pkb, kbmax, pw0, pprobs, pa, psums)

                # one fast-reciprocal covers both heads' sums rows (0 and 64);
                # partitions 1..63 are unwritten psum junk and never read
                recf = sbRC.tile([65, TQ], f32, tag="recf", name="recf")
                ve.reciprocal_approx_fast(recf[:], psums[0:65, :])
                recb = sbRC.tile([65, TQ], bf16, tag="recb", name="recb")
                with nc.allow_low_precision(reason="softmax recip bf16"):
                    ve.tensor_copy(recb[:], recf[:])
                for i, h in enumerate(hs):
                    # broadcast recip across partitions via K=1 matmul
                    pbc = psF.tile([P, TQ], f32, tag="fat", name="fat")
                    te.matmul(pbc[:], onesc[64 * i:64 * i + 1, 0:P],
                              recb[64 * i:64 * i + 1, :],
                              start=True, stop=True)
                    bc = sbBC.tile([P, TQ], bf16, tag="rbc", name="rbc")
                    sc.activation(bc[:], pbc[:], Copy)
                    at = sbAT.tile([P, TQ], bf16, tag=f"attnT{h}",
                                   name=f"attnT{h}")
                    ve.tensor_tensor(at[:], pa[i][:], bc[:], mult)
                    attn_cur[h] = at

        def outproj(gq):
            for tb in range(4):
                ysb = sbY.tile([P, C], bf16, tag="ysb", name="ysb")
                for cc4 in range(4):
                    py = psF.tile([P, 512], f32, tag="fat", name="fat")
                    for h in range(HL):
                        te.matmul(
                            py[:],
                            attn_cur[h][:, tb * P:(tb + 1) * P],
                            woTs[h][:, cc4 * 512:(cc4 + 1) * 512],
                            start=(h == 0), stop=(h == HL - 1),
                        )
                    ve.tensor_copy(ysb[:, cc4 * 512:(cc4 + 1) * 512], py[:])
                sy.dma_start(y_dram[gq][tb * P:(tb + 1) * P, :], ysb[:])
            for hf in range(2):
                gp.collective_compute(
                    "ReduceScatter", mybir.AluOpType.add,
                    replica_groups=[[0, 1, 2, 3], [4, 5, 6, 7]],
                    ins=[y_dram[gq][256 * hf:256 * (hf + 1), :].opt()],
                    outs=[rs_out[2 * gq + hf].opt()],
                )
                sy.dma_start(
                    out.ap()[gq * P + 64 * hf:gq * P + 64 * (hf + 1), :],
                    rs_out[2 * gq + hf][:])

        # ---- pipelined schedule -------------------------------------------
        proj(0)
        late_loads()
        proj(1)
        attn(0)
        outproj(0)
        attn(1)
        outproj(1)
        proj(2)
        attn(2)
        outproj(2)
        proj(3)
        attn(3)
        outproj(3)

    return nc


_PROGRAM = None


def _get_program():
    global _PROGRAM
    if _PROGRAM is None:
        from contextlib import ExitStack
        import concourse.tile as tile
        from concourse import bacc, mybir

        nc = bacc.Bacc("TRN2", target_bir_lowering=False, debug=False,
                       num_devices=NCORES)
        _emit(nc, tile, mybir, ExitStack)
        nc.compile()
        _PROGRAM = nc
    return _PROGRAM


def _bf16(a):
    from ml_dtypes import bfloat16
    return np.asarray(a, np.float32).astype(bfloat16)


def _perm_eo(w):
    """Per 128-row block: rows -> [even rows (64), odd rows (64)]."""
    n = w.shape[0] // P
    w = w.reshape(n, 64, 2, w.shape[-1])
    return np.concatenate([w[:, :, 0, :], w[:, :, 1, :]], axis=1).reshape(
        n * P, -1)


def make_in_maps(x, wq, wk, wv, wo, freqs_cos, freqs_sin):
    x = np.asarray(x, np.float32)
    cos = np.asarray(freqs_cos, np.float32)
    sin = np.asarray(freqs_sin, np.float32)

    cosT = cos.T                      # [64, T]
    sinT = sin.T
    ak = _bf16(np.ascontiguousarray(np.vstack([cosT, cosT])))
    bk = _bf16(np.ascontiguousarray(np.vstack([-sinT, sinT])))
    aq = _bf16(SCALE * np.vstack([cosT, cosT]))
    bq = _bf16(SCALE * np.vstack([-sinT, sinT]))

    xt4s = []
    for b in range(B):
        xt = _bf16(x[b]).T            # [C, T]
        xt4 = np.ascontiguousarray(
            xt.reshape(C, NCH, 512).transpose(1, 0, 2)).reshape(NCH * C, 512)
        xt4s.append(xt4)

    # device constants: causal mask (scoresT layout), identity, ones
    tri = np.where(np.arange(P)[None, :] >= np.arange(P)[:, None],
                   np.float32(0.0), np.float32(NEG)).astype(np.float32)
    cstb = np.zeros((P, 2 * P + 2), np.float32)
    cstb[:, 0:P] = np.eye(P, dtype=np.float32)
    cstb[:, P:] = 1.0
    cstb = _bf16(cstb)

    in_maps = []
    for core in range(NCORES):
        b, g = core // 4, core % 4
        wq_g = _perm_eo(np.asarray(wq[g * HL * D:(g + 1) * HL * D], np.float32))
        wk_g = _perm_eo(np.asarray(wk[g * D:(g + 1) * D], np.float32))
        wv_g = np.asarray(wv[g * D:(g + 1) * D], np.float32)
        wqkv_g = _bf16(np.ascontiguousarray(
            np.concatenate([wq_g.T, wk_g.T, wv_g.T], axis=1)))
        wot_g = _bf16(np.ascontiguousarray(
            np.asarray(wo, np.float32)[:, g * HL * D:(g + 1) * HL * D].T))
        in_maps.append({
            "xt4": xt4s[b],
            "wqkv": wqkv_g,
            "wot": wot_g,
            "raq": aq, "rbq": bq, "rak": ak, "rbk": bk,
            "cstf": tri, "cstb": cstb,
        })
    return in_maps


def kernel(x, wq, wk, wv, wo, freqs_cos, freqs_sin, mask=None):
    from concourse.bass_utils import run_bass_kernel_spmd

    nc = _get_program()
    in_maps = make_in_maps(x, wq, wk, wv, wo, freqs_cos, freqs_sin)
    res = run_bass_kernel_spmd(nc, in_maps, core_ids=list(range(NCORES)))
    outp = np.empty((B, T, C), np.float32)
    for b in range(B):
        for r in range(4):
            piece = np.asarray(res.results[4 * b + r]["out"],
                               dtype=np.float32)  # [NG*128, C]
            for gq in range(NG):
                for hf in range(2):
                    dst = 512 * gq + 256 * hf + 64 * r
                    srow = 128 * gq + 64 * hf
                    outp[b, dst:dst + 64] = piece[srow:srow + 64]
    return outp
